# revision 64
# baseline (speedup 1.0000x reference)
"""Distributed Trainium2 Bass kernel for nn_AttentionCell (B=1, S=4096, D=1024, H=16).

Sharding: tensor-parallel over heads, 2 heads per core, paired (h, h+8) so RoPE's
rotate-half (which pairs model dims d and d+512, i.e. heads h and h+8) stays local
to a core. Per core:
  - QKV projection for its 128 channels (computed transposed: [ch, S]) from a
    host-staged transposed bf16 copy of x.
  - RoPE via host-staged cos/sin tables fused with the bias-add on the
    PSUM->SBUF drain.
  - Causal attention with scores computed transposed ([k, q] layout) so the
    PV matmul needs no on-chip transposes; softmax without max-subtraction
    (scores are O(1) here, exp cannot overflow); denominator via a ones-column
    appended to V (PV output row 64).
  - AllToAll to switch from head-parallel to sequence-parallel, then the
    output projection + bias + LayerNorm on this core's S/8 rows.
Host gathers the 8 per-core row-slices into the full output.
"""
import os
import sys

sys.path.insert(0, "/opt/trn_rl_repo")

import numpy as np
import ml_dtypes

BF = ml_dtypes.bfloat16

DIM = 1024
H = 16
NCORES = 8
QB = 512          # query block (columns of transposed scores)
KT = 128          # key tile (partition dim of transposed scores)
NDIAG = QB // KT  # k-tiles crossing the causal diagonal per q block
LN_EPS = 1e-5
ROPE_THETA = 10000.0


_built = {}


def _build(S):
    """Build + compile the 8-core SPMD graph for sequence length S."""
    import concourse.bass as bass
    import concourse.bacc as bacc
    import concourse.tile as tile
    import concourse.mybir as mybir

    f32 = mybir.dt.float32
    bf16 = mybir.dt.bfloat16
    AF = mybir.ActivationFunctionType
    OP = mybir.AluOpType

    assert S % 512 == 0 and (S // NCORES) % 128 == 0
    SLC = S // NCORES          # output rows per core
    NQB = S // QB              # number of query blocks
    NKT = S // KT              # number of key tiles
    NCH = S // 512             # 512-wide chunks for projections

    nc = bacc.Bacc("TRN2", target_bir_lowering=False, debug=False, num_devices=NCORES)

    TS = max(SLC, QB)          # ctxT tile width (per output-slice staging)

    xt_d = nc.dram_tensor("xt", [DIM, S], bf16, kind="ExternalInput").ap()
    wq_d = nc.dram_tensor("wq", [DIM, 128], bf16, kind="ExternalInput").ap()
    wk_d = nc.dram_tensor("wk", [DIM, 128], bf16, kind="ExternalInput").ap()
    wv_d = nc.dram_tensor("wv", [DIM, 128], bf16, kind="ExternalInput").ap()
    b3_d = nc.dram_tensor("b3", [3, 128, 1], f32, kind="ExternalInput").ap()
    cs_d = nc.dram_tensor("cs", [2, 128, S], bf16, kind="ExternalInput").ap()
    msk_d = nc.dram_tensor("msk", [NDIAG, 128, QB], bf16, kind="ExternalInput").ap()
    id_d = nc.dram_tensor("ident", [128, 128], bf16, kind="ExternalInput").ap()
    wo_d = nc.dram_tensor("wo", [DIM, DIM], bf16, kind="ExternalInput").ap()
    bo16_d = nc.dram_tensor("bo16", [1, DIM], bf16, kind="ExternalInput").ap()
    lnc_d = nc.dram_tensor("lnc", [3, 128, DIM], f32, kind="ExternalInput").ap()
    out_d = nc.dram_tensor("out", [SLC, DIM], f32, kind="ExternalOutput").ap()

    with tile.TileContext(nc) as tc:
        with (
            tc.tile_pool(name="const", bufs=1) as cp,
            tc.tile_pool(name="dram", bufs=1, space="DRAM") as dramp,
        ):
            wq = cp.tile([128, 8, 128], bf16)
            wk = cp.tile([128, 8, 128], bf16)
            wv = cp.tile([128, 8, 128], bf16)
            b3 = cp.tile([128, 3], f32)
            msk = cp.tile([128, NDIAG, QB], bf16)
            ident = cp.tile([128, 128], bf16)
            # per-chunk projection outputs so attention for q-block qb can start
            # as soon as chunks <= qb//2 are projected (causality guarantees the
            # k/v range needed never exceeds the q chunk)
            q_sbs = [cp.tile([128, 512], bf16, name=f"qsb{c}") for c in range(NCH)]
            k_sbs = [cp.tile([128, 512], bf16, name=f"ksb{c}") for c in range(NCH)]
            # [V_A(64) | ones(64) | V_B(64) | ones(64)] per k-tile: the ones block
            # makes the PV matmul emit the softmax denominator broadcast across
            # partitions 64:128 of the ctx accumulator.
            v_alls = [cp.tile([128, 256], bf16, name=f"vall{s}") for s in range(NKT)]
            # per-output-slice ctx staging so each AllToAll bounce DMA can fire
            # as soon as its slice is done
            ctxTs = [cp.tile([128, TS], bf16, name=f"ctxT{j}")
                     for j in range(S // TS)]

            for t in range(8):
                nc.sync.dma_start(wq[:, t, :], wq_d[128 * t:128 * (t + 1), :])
                nc.sync.dma_start(wk[:, t, :], wk_d[128 * t:128 * (t + 1), :])
                nc.sync.dma_start(wv[:, t, :], wv_d[128 * t:128 * (t + 1), :])
            for i in range(3):
                nc.sync.dma_start(b3[:, i:i + 1], b3_d[i])
            for r_ in range(NDIAG):
                nc.sync.dma_start(msk[:, r_, :], msk_d[r_])
            nc.sync.dma_start(ident[:], id_d[:])
            for s_ in range(NKT):
                nc.vector.memset(
                    v_alls[s_][:].rearrange("p (g c) -> p g c", c=64)[:, 1:4:2, :], 1.0)
            epsc = cp.tile([128, 1], f32)
            nc.vector.memset(epsc[:], LN_EPS)
            ones1 = cp.tile([1, 128], bf16)
            nc.vector.memset(ones1[:], 1.0)
            bo16 = cp.tile([1, DIM], bf16)
            nc.sync.dma_start(bo16[:], bo16_d[:])

            # ───── streamed QKV-projection + attention, chunk-interleaved ───
            # For each 512-column chunk ch: project k/q/v and transpose v, then
            # immediately emit attention for q-blocks 2ch and 2ch+1 (causality
            # means their k/v ranges only touch chunks <= ch). Projection,
            # exp, and attention matmuls all overlap in the steady state.
            with (
                tc.tile_pool(name="p1", bufs=1) as p1,
                tc.tile_pool(name="p2", bufs=1) as p2,
                tc.tile_pool(name="ps_mix", bufs=3, space="PSUM") as psmix,
                tc.tile_pool(name="ps_ctx", bufs=1, space="PSUM") as psctx,
            ):
                xtc = [p1.tile([128, 8, 512], bf16, name=f"xtc{c}") for c in range(NCH)]
                csc = [p1.tile([128, 2, 512], bf16, name=f"csc{c}") for c in range(NCH)]
                vts = [p1.tile([128, 512], bf16, name=f"vts{c}") for c in range(NCH)]
                for c in range(NCH):
                    for t in range(8):
                        nc.sync.dma_start(
                            xtc[c][:, t, :],
                            xt_d[128 * t:128 * (t + 1), 512 * c:512 * (c + 1)])
                    for i in range(2):
                        nc.sync.dma_start(
                            csc[c][:, i, :], cs_d[i, :, 512 * c:512 * (c + 1)])

                def proj_chunk(w_sb, b_i, dst, isrope, ch):
                    ps = psmix.tile([128, 512], f32, tag="sc",
                                    padded_shape=[128, 2 * QB], name="psproj")
                    for t in range(8):
                        nc.tensor.matmul(
                            ps[:], w_sb[:, t, :], xtc[ch][:, t, :],
                            start=(t == 0), stop=(t == 7))
                    if isrope:
                        # rot = (p+b)*cos_dup + (p_swapped+b)*sin_signed
                        mA = p1.tile([128, 512], f32, tag="mA", bufs=2)
                        mB = p1.tile([128, 512], f32, tag="mB", bufs=2)
                        bq0 = b3[0:64, b_i:b_i + 1]
                        bq1 = b3[64:128, b_i:b_i + 1]
                        nc.vector.scalar_tensor_tensor(
                            mA[:], ps[:], b3[:, b_i:b_i + 1], csc[ch][:, 0, :],
                            op0=OP.add, op1=OP.mult)
                        nc.vector.scalar_tensor_tensor(
                            mB[0:64, :], ps[64:128, :], bq1, csc[ch][64:128, 1, :],
                            op0=OP.add, op1=OP.mult)
                        nc.vector.scalar_tensor_tensor(
                            mB[64:128, :], ps[0:64, :], bq0, csc[ch][0:64, 1, :],
                            op0=OP.add, op1=OP.mult)
                        nc.vector.tensor_add(dst[:], mA[:], mB[:])
                    else:
                        nc.vector.tensor_scalar_add(vts[ch][:], ps[:], b3[:, 2:3])

                def transpose_chunk(ch):
                    for j in range(4):
                        st = 4 * ch + j
                        tp = psmix.tile([128, 128], bf16, tag="sc",
                                        padded_shape=[128, 4 * QB], name="pstr")
                        nc.tensor.transpose(
                            tp[:], vts[ch][:, 128 * j:128 * (j + 1)], ident[:])
                        nc.vector.tensor_copy(
                            v_alls[st][:].rearrange("p (g c) -> p g c", c=64)[:, 0:4:2, :],
                            tp[:].rearrange("p (g c) -> p g c", c=64))

                def emit_qk(qb, kt):
                    # one group = one 128-wide k-tile against the full 512-wide
                    # q block; head A scores land in bank 0 of the sc slot,
                    # head B in bank 1 (adjacent row-group matmuls must not
                    # share a PSUM bank)
                    rhsA = q_sbs[qb][0:64, :]
                    rhsB = q_sbs[qb][64:128, :]
                    kch, ko = kt // 4, 128 * (kt % 4)
                    sc = psmix.tile([128, 2, QB], f32, tag="sc",
                                    padded_shape=[128, 2, QB], name="scsc")
                    pt = p2.tile([128, 2, QB], bf16, tag="pt", bufs=4)
                    nc.tensor.matmul(
                        sc[:, 0, :], k_sbs[kch][0:64, ko:ko + 128], rhsA,
                        start=True, stop=True)
                    nc.tensor.matmul(
                        sc[:, 1, :], k_sbs[kch][64:128, ko:ko + 128], rhsB,
                        start=True, stop=True)
                    nc.scalar.activation(pt[:], sc[:], AF.Exp, scale=0.125)
                    return pt

                def emit_pv(qb, kt, first, last, pt, ctx):
                    nk = (QB * (qb + 1)) // KT
                    r = kt - NDIAG * qb
                    if r >= 0:  # diagonal-crossing k-tile: apply causal mask
                        nc.vector.tensor_mul(pt[:, 0, :], pt[:, 0, :], msk[:, r, :])
                        nc.vector.tensor_mul(pt[:, 1, :], pt[:, 1, :], msk[:, r, :])
                    nc.tensor.matmul(
                        ctx[:, 0, :], v_alls[kt][:, 0:128], pt[:, 0, :],
                        start=(kt == 0), stop=(kt == nk - 1))
                    nc.tensor.matmul(
                        ctx[:, 1, :], v_alls[kt][:, 128:256], pt[:, 1, :],
                        start=(kt == 0), stop=(kt == nk - 1))

                def emit_norm(qb, ctx):
                    q0 = QB * qb
                    dst = ctxTs[q0 // TS]
                    qs2 = slice(q0 % TS, q0 % TS + QB)
                    rb = p2.tile([64, 2, QB], f32, tag="rb", bufs=2)
                    nc.vector.reciprocal(rb[:], ctx[64:128, :, :])
                    nc.vector.tensor_mul(dst[0:64, qs2], ctx[0:64, 0, :], rb[:, 0, :])
                    nc.vector.tensor_mul(dst[64:128, qs2], ctx[0:64, 1, :], rb[:, 1, :])

                # two-group lookahead: PE stream ... QK(g+1), QK(g+2), PV(g) ...
                from collections import deque
                pending = deque()   # (qb, kt, first, last, pt, ctx)
                ctx = None
                for ch in range(NCH):
                    proj_chunk(wk, 1, k_sbs[ch], True, ch)
                    proj_chunk(wq, 0, q_sbs[ch], True, ch)
                    proj_chunk(wv, 2, None, False, ch)
                    transpose_chunk(ch)
                    qb = ch
                    nk = (QB * (qb + 1)) // KT
                    for kt in range(nk):
                        if kt == 0:
                            ctx = psctx.tile([128, 2, QB], f32, tag="ctx",
                                             padded_shape=[128, 2, QB])
                        pt = emit_qk(qb, kt)
                        pending.append((qb, kt, kt == 0, kt == nk - 1, pt, ctx))
                        if len(pending) > 2:
                            d = pending.popleft()
                            emit_pv(*d)
                            if d[3]:
                                emit_norm(d[0], d[5])
                while pending:
                    d = pending.popleft()
                    emit_pv(*d)
                    if d[3]:
                        emit_norm(d[0], d[5])

            # ───────────────────────── phase 3: AllToAll ────────────────────
            a2a_in = dramp.tile([NCORES, 128, SLC], bf16)
            a2a_out = dramp.tile([NCORES, 128, SLC], bf16)
            for j in range(NCORES):
                src = ctxTs[(SLC * j) // TS]
                c0 = (SLC * j) % TS
                nc.gpsimd.dma_start(a2a_in[j], src[:, c0:c0 + SLC])
            nc.gpsimd.collective_compute(
                "AllToAll",
                mybir.AluOpType.bypass,
                replica_groups=[list(range(NCORES))],
                ins=[a2a_in[:].opt()],
                outs=[a2a_out[:].opt()],
            )

            # ───────────────────────── phase 4: Wo + LayerNorm ──────────────
            with (
                tc.tile_pool(name="p4", bufs=1) as p4,
                tc.tile_pool(name="ps_o", bufs=2, space="PSUM") as pso,
            ):
                wo = p4.tile([128, 8, DIM], bf16)
                lnc = p4.tile([128, 3, DIM], f32)
                ctxF = p4.tile([128, NCORES, SLC], bf16)
                for t in range(8):
                    nc.sync.dma_start(wo[:, t, :], wo_d[128 * t:128 * (t + 1), :])
                for i in range(3):
                    nc.sync.dma_start(lnc[:, i, :], lnc_d[i])
                for j in range(NCORES):
                    nc.sync.dma_start(ctxF[:, j, :], a2a_out[j])

                for qt in range(SLC // 128):
                    tsl = slice(128 * qt, 128 * (qt + 1))
                    ops = pso.tile([128, DIM], f32, tag="o")
                    for nch in range(DIM // 512):
                        osl = slice(512 * nch, 512 * (nch + 1))
                        for ct in range(8):
                            nc.tensor.matmul(
                                ops[:, osl], ctxF[:, ct, tsl], wo[:, ct, osl],
                                start=(ct == 0), stop=False)
                        # rank-1 bias add: ones(128) x bo_row accumulates bo
                        # into every output row, closing the PSUM group
                        nc.tensor.matmul(
                            ops[:, osl], ones1[:], bo16[:, osl],
                            start=False, stop=True)
                    stats = p4.tile([128, 2, 6], f32, tag="stats", bufs=2)
                    nc.vector.bn_stats(stats[:, 0, :], ops[:, 0:512])
                    nc.vector.bn_stats(stats[:, 1, :], ops[:, 512:1024])
                    mv = p4.tile([128, 2], f32, tag="mv", bufs=2)
                    nc.vector.bn_aggr(mv[:], stats[:])
                    sd = p4.tile([128, 2], f32, tag="sd", bufs=2)
                    nc.scalar.activation(sd[:, 0:1], mv[:, 1:2], AF.Sqrt, bias=epsc[:])
                    nc.vector.reciprocal(sd[:, 1:2], sd[:, 0:1])
                    t2 = p4.tile([128, DIM], f32, tag="t2", bufs=2)
                    nc.vector.tensor_scalar(
                        t2[:], ops[:], mv[:, 0:1], sd[:, 1:2],
                        op0=OP.subtract, op1=OP.mult)
                    t3 = p4.tile([128, DIM], f32, tag="t3", bufs=2)
                    nc.vector.tensor_mul(t3[:], t2[:], lnc[:, 1, :])
                    ob = p4.tile([128, DIM], f32, tag="ob", bufs=2)
                    nc.vector.tensor_add(ob[:], t3[:], lnc[:, 2, :])
                    nc.sync.dma_start(out_d[tsl, :], ob[:])

    nc.compile()
    return nc


def get_nc(S=4096):
    if S not in _built:
        _built[S] = _build(S)
    return _built[S]


def stage_inputs(x, Wqkv, bqkv, Wo, bo, gamma, beta):
    """Host-side sharding/staging. Returns in_maps for the 8 cores."""
    x = np.asarray(x, dtype=np.float32)
    Wqkv = np.asarray(Wqkv, dtype=np.float32)
    bqkv = np.asarray(bqkv, dtype=np.float32)
    Wo = np.asarray(Wo, dtype=np.float32)
    bo = np.asarray(bo, dtype=np.float32)
    gamma = np.asarray(gamma, dtype=np.float32)
    beta = np.asarray(beta, dtype=np.float32)

    S = x.shape[1]
    xt = np.ascontiguousarray(x[0].T).astype(BF)                       # [DIM, S]
    inv_freq = 1.0 / (ROPE_THETA ** (np.arange(0, DIM, 2, dtype=np.float64) / DIM))

    # Wo rows permuted to the post-AllToAll channel order
    perm = np.concatenate([
        np.concatenate([np.arange(64 * j, 64 * j + 64),
                        np.arange(512 + 64 * j, 512 + 64 * j + 64)])
        for j in range(NCORES)
    ])
    wo = np.ascontiguousarray(Wo[perm, :]).astype(BF)

    p = np.arange(128)[:, None]
    f = np.arange(QB)[None, :]
    # per-diagonal-k-tile causal masks: msk[r][p, f] = (128*r + p <= f)
    msk = np.stack([(128 * r + p <= f) for r in range(NDIAG)]).astype(BF)
    ident = np.eye(128, dtype=np.float32).astype(BF)
    lnc = np.stack([
        np.broadcast_to(bo, (128, DIM)),
        np.broadcast_to(gamma, (128, DIM)),
        np.broadcast_to(beta, (128, DIM)),
    ]).astype(np.float32)

    in_maps = []
    for c in range(NCORES):
        cols = np.concatenate([np.arange(64 * c, 64 * c + 64),
                               np.arange(512 + 64 * c, 512 + 64 * c + 64)])
        ang = np.arange(S, dtype=np.float64)[None, :] * inv_freq[64 * c:64 * c + 64][:, None]
        C = np.cos(ang)
        Sn = np.sin(ang)
        # plane 0: cos duplicated; plane 1: +sin rows 0:64, -sin rows 64:128
        # (the sign flip folds the rotate-half subtraction into one tensor_add)
        cs = np.stack([np.concatenate([C, C], 0),
                       np.concatenate([Sn, -Sn], 0)]).astype(BF)        # [2,128,S]
        b3 = np.stack([bqkv[cols], bqkv[1024 + cols], bqkv[2048 + cols]]
                      ).astype(np.float32)[:, :, None]                  # [3,128,1]
        in_maps.append({
            "xt": xt,
            "wq": np.ascontiguousarray(Wqkv[:, cols]).astype(BF),
            "wk": np.ascontiguousarray(Wqkv[:, 1024 + cols]).astype(BF),
            "wv": np.ascontiguousarray(Wqkv[:, 2048 + cols]).astype(BF),
            "b3": b3,
            "cs": cs,
            "msk": msk,
            "ident": ident,
            "wo": wo,
            "bo16": bo.reshape(1, DIM).astype(BF),
            "lnc": lnc,
        })
    return in_maps


def kernel(x, Wqkv, bqkv, Wo, bo, gamma, beta):
    from concourse import bass_utils

    x = np.asarray(x)
    S = x.shape[1]
    nc = get_nc(S)
    in_maps = stage_inputs(x, Wqkv, bqkv, Wo, bo, gamma, beta)
    res = bass_utils.run_bass_kernel_spmd(nc, in_maps, core_ids=list(range(NCORES)))
    out = np.concatenate([res.results[c]["out"] for c in range(NCORES)], axis=0)
    return out[None].astype(np.float32)
